# revision 60
# baseline (speedup 1.0000x reference)
"""Trainium2 Bass kernel for nn_Adapter (audio conv encoder + cross-attention), v4.

Baseline pipeline + surgical wins:
  - permuted e'-layout [c0: pair0+pair1a | c1: pair2+pair1b | c2: pair3]:
    sim 6->5 MMs, AV 9->8 MMs per chunk; denominators at psum rows 96:104
  - audio convs col-tiled (two concurrent MMs on array col-halves)
  - bf16 output; audio-first const loads; xb2/xb3 assembly DMAs on scalar queue
"""
import sys
sys.path.insert(0, "/opt/trn_rl_repo")

import numpy as np
import ml_dtypes

import concourse.bass as bass
import concourse.mybir as mybir
import concourse.tile as tile
from concourse.bass_utils import run_bass_kernel_spmd

F32 = mybir.dt.float32
BF16 = mybir.dt.bfloat16
AF = mybir.ActivationFunctionType
BF = ml_dtypes.bfloat16

NCORES = 8
B, N, CTX = 16, 4096, 768
BP = B // NCORES
H, D, INNER = 8, 40, 320
AUD = 1024
KS, PAD = 17, 8
EPS = 1e-5
SCALE = D ** -0.5
TCH = 512
NCH = N // TCH
PADB = AUD + 2 * PAD

# e' permutation: slot -> old e (-1 = pad); bias at slot 352 (chunk2 row 96)
def _perm_new2old():
    p = [-1] * 384
    for s in range(128):
        p[s] = s                    # c0: pair0 (0:80) + pair1 dims 0:48
    for i in range(80):
        p[128 + i] = 160 + i        # c1 rows 0:80: pair2
    for i in range(32):
        p[208 + i] = 128 + i        # c1 rows 80:112: pair1 dims 48:80
    for i in range(80):
        p[256 + i] = 240 + i        # c2 rows 0:80: pair3
    return p


PN2O = _perm_new2old()
RN = [128, 112, 97]      # out-proj contraction rows (96=bias)
ME = [128, 112, 80]      # value rows per e'-chunk (normalize extent)


def _head_of_slot(n, r):
    o = PN2O[128 * n + r]
    return -1 if o < 0 else o // D


def _head_runs():
    """Contiguous (n, r0, r1, head) runs over the e'-chunks' value rows."""
    runs = []
    for n in range(3):
        r = 0
        while r < ME[n]:
            h = _head_of_slot(n, r)
            r1 = r
            while r1 < ME[n] and _head_of_slot(n, r1) == h:
                r1 += 1
            runs.append((n, r, r1, h))
            r = r1
    return runs


HEAD_RUNS = _head_runs()


def _build_host_consts(inputs):
    c = {}
    w1, b1 = inputs["w1"], inputs["b1"]
    w2, b2 = inputs["w2"], inputs["b2"]
    w3, b3 = inputs["w3"], inputs["b3"]
    c["w1t"] = np.ascontiguousarray(w1[:, 0, :].T).astype(BF)

    def pack_pairs(w):
        wp = np.zeros((128, 9, 64), np.float32)
        wt = w.transpose(1, 2, 0)
        for q in range(9):
            wp[0:64, q, :] = wt[:, 2 * q, :]
            if 2 * q + 1 < KS:
                wp[64:128, q, :] = wt[:, 2 * q + 1, :]
        return wp.astype(BF)

    c["w2p"] = pack_pairs(w2)
    c["w3p"] = pack_pairs(w3)

    def dup2(v):
        return np.tile(np.asarray(v), 2).reshape(128, 1).astype(np.float32)

    c["b1c2"] = dup2(b1)
    c["b2c2"] = dup2(b2)
    c["b3c2"] = dup2(b3)
    lnw = np.asarray(inputs["ln_w"]).astype(np.float32)
    lnb = np.asarray(inputs["ln_b"]).astype(np.float32)
    c["lnw2"] = np.concatenate([lnw[:, 0:512], lnw[:, 512:1024]], 0)
    c["lnb2"] = np.concatenate([lnb[:, 0:512], lnb[:, 512:1024]], 0)

    wq = np.asarray(inputs["wq"])
    wk = np.asarray(inputs["wk"])
    wv = np.asarray(inputs["wv"])
    wout = np.asarray(inputs["w_out"])
    bout = np.asarray(inputs["b_out"])
    wqt = np.zeros((CTX, 384), np.float32)
    wkt = np.zeros((AUD, 384), np.float32)
    wvt = np.zeros((AUD, 384), np.float32)
    wA = np.zeros((384, CTX), np.float32)
    for s, o in enumerate(PN2O):
        if o >= 0:
            wqt[:, s] = wq[o, :]
            wkt[:, s] = wk[o, :]
            wvt[:, s] = wv[o, :]
            wA[s] = wout[:, o]
    wA[352] = bout
    # pre-permute to [128p, n, e] so DMA reads are fully contiguous
    c["wqt"] = np.ascontiguousarray(
        wqt.reshape(6, 128, 384).transpose(1, 0, 2)).astype(BF)
    c["wkt"] = np.ascontiguousarray(
        wkt.reshape(8, 128, 384).transpose(1, 0, 2)).astype(BF)
    c["wvt"] = np.ascontiguousarray(
        wvt.reshape(8, 128, 384).transpose(1, 0, 2)).astype(BF)
    c["woutA"] = np.ascontiguousarray(
        wA.reshape(3, 128, CTX).transpose(1, 0, 2)).astype(BF)

    km_lo = np.zeros((80, 128), np.float32)
    km_lo[0:40, 0:64] = 1.0
    km_lo[40:80, 64:128] = 1.0
    c["kmask_lo"] = km_lo.astype(BF)
    km1a = np.zeros((128, 128), np.float32)
    km1a[80:120, 0:64] = 1.0
    km1a[120:128, 64:128] = 1.0
    c["kmask1a"] = km1a.astype(BF)
    km1b = np.zeros((128, 128), np.float32)
    km1b[80:112, 64:128] = 1.0
    c["kmask1b"] = km1b.astype(BF)

    vm = np.zeros((128, 80), np.float32)
    vm[0:64, 0:40] = 1.0
    vm[64:128, 40:80] = 1.0
    c["vmask_v"] = vm.astype(BF)

    # exp8': [104, 3, 128] head->row broadcast selector (rows 96:104 used)
    e8 = np.zeros((104, 3, 128), np.float32)
    for n in range(3):
        for r in range(ME[n]):
            h = _head_of_slot(n, r)
            if h >= 0:
                e8[96 + h, n, r] = 1.0
    c["exp8"] = e8.astype(BF)

    # mini-blobs: conv1 weights (bf16) + the three biases (f32)
    mini = np.zeros((17, 64), BF)
    mini[:, :] = c["w1t"]
    c["mini"] = mini
    minif = np.concatenate([c.pop("b1c2"), c.pop("b2c2"), c.pop("b3c2")], 1)
    c["minif"] = np.ascontiguousarray(minif)
    # pack small consts into two blobs: one DMA each at kernel start
    bb = np.zeros((128, 2064), BF)
    bb[0:17, 0:64] = c.pop("w1t")
    bb[:, 64:640] = c.pop("w2p").reshape(128, 576)
    bb[:, 640:1216] = c.pop("w3p").reshape(128, 576)
    bb[0:80, 1216:1344] = c.pop("kmask_lo")
    bb[:, 1344:1472] = c.pop("kmask1a")
    bb[:, 1472:1600] = c.pop("kmask1b")
    bb[:, 1600:1680] = c.pop("vmask_v")
    bb[0:104, 1680:2064] = c.pop("exp8").reshape(104, 384)
    c["blob_bf"] = bb
    bf = np.zeros((128, 1155), np.float32)
    bf[:, 3:515] = c.pop("lnw2")
    bf[:, 515:1027] = c.pop("lnb2")
    bf[:, 1027:1155] = np.eye(128, dtype=np.float32)
    c["blob_f"] = bf
    return c


def _build_graph():
    nc = bass.Bass()
    P = {}

    def inp(name, shape, dt):
        P[name] = nc.declare_dram_parameter(name, list(shape), dt, isOutput=False)

    inp("ctx16", (BP, NCH, 128, 6, TCH), BF16)
    inp("a_im", (BP, KS, AUD), BF16)
    inp("blob_bf", (128, 2064), BF16)
    inp("blob_f", (128, 1155), F32)
    inp("wqt", (128, 6, 384), BF16)
    inp("wkt", (128, 8, 384), BF16)
    inp("wvt", (128, 8, 384), BF16)
    inp("woutA", (128, 3, CTX), BF16)
    out_e = nc.declare_dram_parameter("out", [BP, N, CTX], BF16, isOutput=True)

    with tile.TileContext(nc) as tc:
        cp = tc.alloc_tile_pool(name="const", bufs=1)
        pp = tc.alloc_tile_pool(name="persist", bufs=1)
        cinp = tc.alloc_tile_pool(name="cinp", bufs=7)
        esp = tc.alloc_tile_pool(name="esp", bufs=6)
        qtp = tc.alloc_tile_pool(name="qtp", bufs=6)
        mp = tc.alloc_tile_pool(name="mp", bufs=2)
        ofp = tc.alloc_tile_pool(name="ofp", bufs=2)
        ap = tc.alloc_tile_pool(name="audio", bufs=1)
        aps = tc.alloc_tile_pool(name="aps", bufs=2, space="PSUM")

        def cload(name, shape, dt, ap_src=None):
            t = cp.tile(list(shape), dt, tag=name)
            nc.sync.dma_start(t[:], ap_src if ap_src is not None else P[name][:])
            return t

        # ---- t0: trigger the gelu ACT-table load while DMAs stream ----
        tiny = cp.tile([1, 2], F32, tag="tiny")
        nc.vector.memset(tiny[:], 0.0)
        tinyg = cp.tile([1, 2], F32, tag="tinyg")
        nc.scalar.activation(tinyg[:], tiny[:], AF.Gelu)

        # ---- mini-blob (conv1 weights + biases) first, audio inputs,
        # ---- then one blob DMA for the remaining small consts ----
        mini = cp.tile([KS, 64], BF16, tag="mini")
        nc.sync.dma_start(mini[:], P["mini"][:])
        w1t = mini[:]
        minif = cp.tile([128, 3], F32, tag="minif")
        nc.sync.dma_start(minif[:], P["minif"][:])
        b1c2 = minif[:, 0:1]
        b2c2 = minif[:, 1:2]
        b3c2 = minif[:, 2:3]

        a_sbs = {}
        for b in range(BP):
            a_sb = ap.tile([KS, AUD], BF16, tag=f"a_im{b}")
            nc.sync.dma_start(a_sb[:], P["a_im"][b])
            a_sbs[b] = a_sb

        ones128 = cp.tile([128, 128], BF16, tag="ones128")
        nc.vector.memset(ones128[:], 1.0)

        # ---- audio encoder phases (dual-row [128, 512] layout) ----
        xb2s, x2bs, statss, xb3s, x_sbs, xts = {}, {}, {}, {}, {}, {}
        g1s = {}
        kp_all, vp_all = [None, None], [None, None]

        def asm_dual(dst, g):
            nc.gpsimd.memset(dst[0:64, 0:PAD], 0.0)
            nc.gpsimd.memset(dst[0:64, AUD + PAD:PADB], 0.0)
            nc.gpsimd.memset(dst[64:128, 0:PAD - 1], 0.0)
            nc.gpsimd.memset(dst[64:128, PAD + AUD - 1:PADB], 0.0)
            nc.scalar.dma_start(dst[0:64, PAD:PAD + 512], g[0:64, :])
            nc.scalar.dma_start(dst[0:64, PAD + 512:PAD + 1024], g[64:128, :])
            nc.scalar.dma_start(dst[64:128, PAD - 1:PAD + 511], g[0:64, :])
            nc.scalar.dma_start(dst[64:128, PAD + 511:PAD + 1023], g[64:128, :])

        def conv_ct(psp, cvtag, wtile, src):
            cv = psp.tile([128, 512], F32, tag=cvtag)
            for q in range(9):
                nc.tensor.matmul(cv[0:64, :], wtile[:, q, :],
                                 src[:, 2 * q:2 * q + 512],
                                 start=(q == 0), stop=(q == 8),
                                 tile_position=(0, 0))
                nc.tensor.matmul(cv[64:128, :], wtile[:, q, :],
                                 src[:, 2 * q + 512:2 * q + 1024],
                                 start=(q == 0), stop=(q == 8),
                                 tile_position=(0, 64))
            return cv

        def ph_conv1(b, psp, cvtag):
            cv1 = psp.tile([128, 512], F32, tag=cvtag)
            nc.tensor.matmul(cv1[0:64, :], w1t[:], a_sbs[b][:, 0:512],
                             start=True, stop=True, tile_position=(0, 0))
            nc.tensor.matmul(cv1[64:128, :], w1t[:], a_sbs[b][:, 512:1024],
                             start=True, stop=True, tile_position=(0, 64))
            g1 = ap.tile([128, 512], BF16, tag=f"g1{b}")
            nc.scalar.activation(g1[:], cv1[:], AF.Gelu, bias=b1c2[:])
            g1s[b] = g1
            xb2 = ap.tile([128, PADB], BF16, tag=f"xb2{b}")
            asm_dual(xb2, g1)
            xb2s[b] = xb2

        def ph_conv2(b, psp, cvtag):
            cv2 = conv_ct(psp, cvtag, w2p, xb2s[b])
            x2b = ap.tile([128, 512], F32, tag=f"x2b{b}")
            stats = ap.tile([128, 2], F32, tag=f"stats{b}")
            sq = ap.tile([128, 512], F32, tag=f"sq{b}")
            nc.vector.tensor_scalar(
                out=x2b[:], in0=cv2[:], scalar1=b2c2[:], scalar2=0.0,
                op0=mybir.AluOpType.add, op1=mybir.AluOpType.add,
                accum_out=stats[:, 0:1])
            nc.vector.tensor_mul(sq[:], x2b[:], x2b[:])
            nc.vector.reduce_sum(stats[:, 1:2], sq[:], axis=mybir.AxisListType.X)
            x2bs[b] = x2b
            statss[b] = stats

        def ph_ln(b, psp, cvtag):
            stats = statss[b]
            x2b = x2bs[b]
            st16 = ap.tile([128, 2], BF16, tag=f"st16{b}")
            nc.vector.tensor_copy(st16[:], stats[:])
            totp = psp.tile([128, 64], F32, tag=cvtag)
            nc.tensor.matmul(totp[:, 0:2], ones128[:], st16[:], start=True, stop=True)

            mu = ap.tile([128, 1], F32, tag=f"mu{b}")
            msq = ap.tile([128, 1], F32, tag=f"msq{b}")
            var = ap.tile([128, 1], F32, tag=f"var{b}")
            sd = ap.tile([128, 1], F32, tag=f"sd{b}")
            rstd = ap.tile([128, 1], F32, tag=f"rstd{b}")
            nmr = ap.tile([128, 1], F32, tag=f"nmr{b}")
            inv_n = 1.0 / (64 * AUD)
            nc.vector.tensor_scalar_mul(mu[:], totp[:, 0:1], inv_n)
            nc.vector.tensor_scalar_mul(msq[:], totp[:, 1:2], inv_n)
            nc.vector.tensor_mul(var[:], mu[:], mu[:])
            nc.vector.tensor_sub(var[:], msq[:], var[:])
            nc.vector.tensor_scalar_add(var[:], var[:], EPS)
            nc.scalar.activation(sd[:], var[:], AF.Ln)
            nc.scalar.activation(rstd[:], sd[:], AF.Exp, scale=-0.5)
            nc.vector.tensor_mul(nmr[:], mu[:], rstd[:])
            nc.vector.tensor_scalar_mul(nmr[:], nmr[:], -1.0)

            t1 = ap.tile([128, 512], F32, tag=f"t1{b}")
            t2 = ap.tile([128, 512], F32, tag=f"t2{b}")
            g3 = ap.tile([128, 512], BF16, tag=f"g3{b}")
            nc.vector.tensor_scalar(out=t1[:], in0=x2b[:], scalar1=rstd[:],
                                    scalar2=nmr[:], op0=mybir.AluOpType.mult,
                                    op1=mybir.AluOpType.add)
            nc.vector.tensor_mul(t2[:], t1[:], lnw2[:])
            nc.vector.tensor_add(g3[:], t2[:], lnb2[:])
            xb3 = ap.tile([128, PADB], BF16, tag=f"xb3{b}")
            asm_dual(xb3, g3)
            xb3s[b] = xb3

        def ph_conv3(b, psp, cvtag):
            cv3 = conv_ct(psp, cvtag, w3p, xb3s[b])
            x_sb = ap.tile([128, 512], F32, tag=f"x_sb{b}")
            nc.vector.tensor_scalar(
                out=x_sb[:], in0=cv3[:], scalar1=b3c2[:], scalar2=0.0,
                op0=mybir.AluOpType.add, op1=mybir.AluOpType.add)
            x_sbs[b] = x_sb

        def ph_xt(b, psp, cvtag):
            # one [128,128] transpose yields both L-halves' [128L, 64ch] tiles
            xt = pp.tile([128, 8, 64], BF16, tag=f"xt{b}")
            for f in range(4):
                pt = psp.tile([128, 128], F32, tag=cvtag)
                nc.tensor.transpose(pt[:], x_sbs[b][:, 128 * f:128 * f + 128],
                                    ident128[:])
                nc.scalar.activation(xt[:, f::4, :], pt[:], AF.Copy)
            xts[b] = xt

        def ph_ktv(b, psp, cvtag):
            xt = xts[b]
            kt = pp.tile([128, 3, 64], BF16, tag=f"kt{b}")
            for m in range(3):
                ktp = psp.tile([128, 64], F32, tag=cvtag)
                for aj in range(8):
                    nc.tensor.matmul(ktp[:], wkt[:, aj, 128 * m:128 * m + 128],
                                     xt[:, aj, :], start=(aj == 0), stop=(aj == 7))
                nc.scalar.activation(kt[:, m, :], ktp[:], AF.Copy)

            v2p = psp.tile([128, 384], F32, tag=cvtag)
            for aj in range(8):
                nc.tensor.matmul(v2p[0:64, :], xt[:, aj, :], wvt[:, aj, :],
                                 start=(aj == 0), stop=(aj == 7),
                                 tile_position=(0, 0))
                nc.tensor.matmul(v2p[64:128, :], xt[:, aj, :], wvt[:, aj, :],
                                 start=(aj == 0), stop=(aj == 7),
                                 tile_position=(0, 64))
            v2 = pp.tile([128, 384], BF16, tag=f"v2{b}")
            nc.scalar.activation(v2[:], v2p[:], AF.Copy)

            # kp statics for sim (5 MMs/chunk)
            def mk_kp80(tag, ktsl):
                t = pp.tile([80, 128], BF16, tag=tag)
                nc.vector.tensor_mul(
                    t[:].rearrange("p (a j) -> p a j", a=2),
                    ktsl.broadcast_to([80, 2, 64]),
                    kmask_lo[:].rearrange("p (a j) -> p a j", a=2))
                return t

            kp0 = mk_kp80(f"kp0_{b}", kt[0:80, 0:1, :])
            kp2 = mk_kp80(f"kp2_{b}", kt[0:80, 1:2, :])
            kp3 = mk_kp80(f"kp3_{b}", kt[0:80, 2:3, :])
            kp1a = pp.tile([128, 128], BF16, tag=f"kp1a_{b}")
            nc.vector.tensor_mul(
                kp1a[64:128, :].rearrange("p (a j) -> p a j", a=2),
                kt[64:128, 0:1, :].broadcast_to([64, 2, 64]),
                kmask1a[64:128, :].rearrange("p (a j) -> p a j", a=2))
            kp1b = pp.tile([128, 128], BF16, tag=f"kp1b_{b}")
            nc.vector.tensor_mul(
                kp1b[64:128, :].rearrange("p (a j) -> p a j", a=2),
                kt[64:128, 1:2, :].broadcast_to([64, 2, 64]),
                kmask1b[64:128, :].rearrange("p (a j) -> p a j", a=2))
            kp_all[b] = (kp0, kp1a, kp1b, kp2, kp3)

            # vp value statics (baseline-style per (chunk, pair))
            vps = {}
            vp = pp.tile([128, 128], BF16, tag=f"vp00_{b}")     # (0, p0)
            nc.gpsimd.memset(vp[:, 80:128], 0.0)
            nc.vector.tensor_mul(vp[:, 0:80], v2[:, 0:80], vmask_v[:])
            vps[(0, 0)] = vp
            vp = pp.tile([128, 128], BF16, tag=f"vp01_{b}")     # (0, p1) dims 0:48
            nc.gpsimd.memset(vp[:, 0:80], 0.0)
            nc.vector.tensor_mul(vp[:, 80:128], v2[:, 80:128], vmask_v[:, 0:48])
            vps[(0, 1)] = vp
            vp = pp.tile([128, 112], BF16, tag=f"vp12_{b}")     # (1, p2)
            nc.gpsimd.memset(vp[:, 80:112], 0.0)
            nc.vector.tensor_mul(vp[:, 0:80], v2[:, 128:208], vmask_v[:])
            vps[(1, 2)] = vp
            vp = pp.tile([128, 112], BF16, tag=f"vp11_{b}")     # (1, p1) dims 48:80
            nc.gpsimd.memset(vp[:, 0:80], 0.0)
            nc.vector.tensor_mul(vp[:, 80:112], v2[:, 208:240], vmask_v[:, 48:80])
            vps[(1, 1)] = vp
            vp = pp.tile([128, 104], BF16, tag=f"vp23_{b}")     # (2, p3) + denoms
            nc.gpsimd.memset(vp[:, 80:104], 0.0)
            nc.vector.tensor_mul(vp[:, 0:80], v2[:, 256:336], vmask_v[:])
            nc.gpsimd.memset(vp[0:64, 102:103], 1.0)
            nc.gpsimd.memset(vp[64:128, 103:104], 1.0)
            vps[(2, 3)] = vp
            vp_all[b] = vps

        # denominator-only statics for at2 (batch-independent)
        vpd = []
        for p in range(3):
            t = cp.tile([128, 104], BF16, tag=f"vpd{p}")
            nc.gpsimd.memset(t[:], 0.0)
            nc.gpsimd.memset(t[0:64, 96 + 2 * p:97 + 2 * p], 1.0)
            nc.gpsimd.memset(t[64:128, 97 + 2 * p:98 + 2 * p], 1.0)
            vpd.append(t)

        # ---- attention loads by need ----
        wqt = cload("wqt", (128, 6, 384), BF16)
        cin_pre = {}
        t = cinp.tile([128, 6, TCH], BF16, tag="cin", name="cin0")
        nc.sync.dma_start(t[:], P["ctx16"][0, 0])
        cin_pre[(0, 0)] = t
        blob_bf = cp.tile([128, 2064], BF16, tag="blob_bf")
        nc.sync.dma_start(blob_bf[:], P["blob_bf"][:])
        w2p = blob_bf[:, 64:640].rearrange("p (a b) -> p a b", a=9)
        w3p = blob_bf[:, 640:1216].rearrange("p (a b) -> p a b", a=9)
        kmask_lo = blob_bf[0:80, 1216:1344]
        kmask1a = blob_bf[:, 1344:1472]
        kmask1b = blob_bf[:, 1472:1600]
        vmask_v = blob_bf[:, 1600:1680]
        exp8 = blob_bf[0:104, 1680:2064].rearrange("p (a b) -> p a b", a=3)

        t = cinp.tile([128, 6, TCH], BF16, tag="cin", name="cin1")
        nc.sync.dma_start(t[:], P["ctx16"][0, 1])
        cin_pre[(0, 1)] = t
        blob_f = cp.tile([128, 1155], F32, tag="blob_f")
        nc.sync.dma_start(blob_f[:], P["blob_f"][:])
        lnw2 = blob_f[:, 3:515]
        lnb2 = blob_f[:, 515:1027]
        ident128 = blob_f[:, 1027:1155]
        wkt = cload("wkt", (128, 8, 384), BF16)
        wvt = cload("wvt", (128, 8, 384), BF16)
        for c0_ in range(2, 4):
            t = cinp.tile([128, 6, TCH], BF16, tag="cin")
            nc.sync.dma_start(t[:], P["ctx16"][0, c0_])
            cin_pre[(0, c0_)] = t
        woutA = cload("woutA", (128, 3, CTX), BF16)
        for c0_ in range(4, 6):
            t = cinp.tile([128, 6, TCH], BF16, tag="cin")
            nc.sync.dma_start(t[:], P["ctx16"][0, c0_])
            cin_pre[(0, c0_)] = t

        def emit_q(cin, psum_pool, psum_tag):
            qt = qtp.tile([128, 3, TCH], BF16, tag="qt")
            for m in range(3):
                qp = psum_pool.tile([128, TCH], F32, tag=psum_tag)
                for n6 in range(6):
                    nc.tensor.matmul(qp[:], wqt[:, n6, 128 * m:128 * m + 128],
                                     cin[:, n6, :], start=(n6 == 0), stop=(n6 == 5))
                nc.vector.tensor_copy(qt[:, m, :], qp[:])
            return qt

        # ---- pre-phase: full audio for both batches + first 4 q-emits ----
        ph_conv1(0, aps, "cv")
        ph_conv1(1, aps, "cv")
        # after both gelus: swap the ACT table to natural_log_exp.
        # reads g1(b1) so the scheduler cannot hoist it before the gelus.
        tinyl = cp.tile([1, 2], F32, tag="tinyl")
        nc.scalar.activation(tinyl[:], g1s[1][0:1, 0:2], AF.Ln)

        qt_pre = {}
        qt_pre[0] = emit_q(cin_pre[(0, 0)], aps, "qpre")
        qt_pre[1] = emit_q(cin_pre[(0, 1)], aps, "qpre")
        ph_conv2(0, aps, "cv")
        ph_conv2(1, aps, "cv")
        qt_pre[2] = emit_q(cin_pre[(0, 2)], aps, "qpre")
        ph_ln(0, aps, "cv")
        ph_ln(1, aps, "cv")
        ph_conv3(0, aps, "cv")
        ph_conv3(1, aps, "cv")
        qt_pre[3] = emit_q(cin_pre[(0, 3)], aps, "qpre")
        ph_xt(0, aps, "cv")
        ph_xt(1, aps, "cv")
        ph_ktv(0, aps, "cv")
        qt_pre[4] = emit_q(cin_pre[(0, 4)], aps, "qpre")
        ph_ktv(1, aps, "cv")
        qt_pre[5] = emit_q(cin_pre[(0, 5)], aps, "qpre")

        aps.release()

        # ---- main attention loop ----
        mps = tc.alloc_tile_pool(name="mps", bufs=2, space="PSUM")

        at_sbs = []
        for k2 in range(2):
            t = pp.tile([128, 3, TCH], BF16, tag=f"at_sb{k2}")
            nc.gpsimd.memset(t[64:96, 2, :], 0.0)
            nc.gpsimd.memset(t[96:97, 2, :], 1.0)
            at_sbs.append(t)

        pending_out = None

        def emit_tt(job, tt, of):
            ob, oc, oat = job
            for ci, (c0, cw) in enumerate(((0, 384), (384, 384))):
                op = mps.tile([128, 512], F32, tag="ob")
                for n in range(3):
                    nc.tensor.matmul(
                        op[:, 0:cw],
                        oat[0:RN[n], n, 128 * tt:128 * tt + 128],
                        woutA[0:RN[n], n, c0:c0 + cw],
                        start=(n == 0), stop=(n == 2))
                if ci == 0:
                    nc.scalar.activation(of[:, tt, c0:c0 + cw], op[:, 0:cw],
                                         AF.Copy)
                else:
                    nc.vector.tensor_copy(of[:, tt, c0:c0 + cw], op[:, 0:cw])

        def emit_dma(job, of):
            ob, oc, oat = job
            nc.sync.dma_start(
                out_e[ob, TCH * oc:TCH * oc + TCH, :]
                .rearrange("(a p) c -> p a c", p=128), of[:])

        chunks = [(bb, cc2) for bb in range(BP) for cc2 in range(NCH)]
        cins = dict(cin_pre)
        qts = {i2: qt_pre[i2] for i2 in range(len(qt_pre))}

        for i, (b, c) in enumerate(chunks):
            kp0, kp1a, kp1b, kp2, kp3 = kp_all[b]
            vps = vp_all[b]
            for la in (3, 4):
                if i + la < len(chunks) and chunks[i + la] not in cins:
                    b3, c3 = chunks[i + la]
                    t = cinp.tile([128, 6, TCH], BF16, tag="cin")
                    nc.gpsimd.dma_start(t[:], P["ctx16"][b3, c3])
                    cins[chunks[i + la]] = t

            qt = qts.pop(i)
            of_cur = (ofp.tile([128, 4, CTX], BF16, tag="of", name="of_cur")
                      if pending_out is not None else None)

            sim_defs = [
                [(kp0[:], qt[0:80, 0, :], None)],
                [(kp1a[64:128, :], qt[64:128, 0, :], (64, 0)),
                 (kp1b[64:128, :], qt[64:128, 1, :], (64, 0))],
                [(kp2[:], qt[0:80, 1, :], None)],
                [(kp3[:], qt[0:80, 2, :], None)],
            ]
            es = [None] * 4

            def sim(p):
                sp = mps.tile([128, TCH], F32, tag="sp")
                plan = sim_defs[p]
                for ii, (lh, rh, tp) in enumerate(plan):
                    nc.tensor.matmul(sp[:], lh, rh, start=(ii == 0),
                                     stop=(ii == len(plan) - 1),
                                     tile_position=tp)
                e = esp.tile([128, TCH], BF16, tag="es")
                nc.scalar.activation(e[:], sp[:], AF.Exp, scale=SCALE)
                es[p] = e

            sim(0)
            sim(1)
            # q for chunk i+2 fills the PE while the first exps run on ACT
            if i + 2 < len(chunks) and (i + 2) not in qts:
                qts[i + 2] = emit_q(cins.pop(chunks[i + 2]), mps, "qp")
            sim(2)
            sim(3)

            at_sb = at_sbs[i % 2]

            # at2: chunk2 AV (pair3) + all denominators at rows 96:104
            at2 = mps.tile([104, TCH], F32, tag="at")
            at2_ops = [(vpd[0], 0), (vpd[1], 1), (vpd[2], 2), (vps[(2, 3)], 3)]
            for ii, (vpt, p) in enumerate(at2_ops):
                nc.tensor.matmul(at2[:], vpt[:, 0:104], es[p][:],
                                 start=(ii == 0), stop=(ii == 3))
            lnd = mp.tile([104, TCH], F32, tag="lnd")
            rec16 = mp.tile([104, TCH], BF16, tag="rec16")
            nc.scalar.activation(lnd[96:104, :], at2[96:104, :], AF.Ln)
            nc.scalar.activation(rec16[96:104, :], lnd[96:104, :], AF.Exp,
                                 scale=-1.0)

            def brs_of(n):
                brp = mps.tile([128, TCH], F32, tag="sp")
                nc.tensor.matmul(brp[0:ME[n], :], exp8[96:104, n, 0:ME[n]],
                                 rec16[96:104, :], start=True, stop=True,
                                 tile_position=(96, 0))
                brs = mp.tile([128, TCH], BF16, tag="brs", bufs=3)
                nc.vector.tensor_copy(brs[0:ME[n], :], brp[0:ME[n], :])
                return brs

            def av(n):
                a = mps.tile([128, TCH], F32, tag="at")
                W = 128 if n == 0 else 112
                prs = [(0, 0), (0, 1)] if n == 0 else [(1, 2), (1, 1)]
                for ii, key in enumerate(prs):
                    nc.tensor.matmul(a[0:W, :], vps[key][:], es[key[1]][:],
                                     start=(ii == 0), stop=(ii == 1))
                return a

            def mul(n, at_ps, brs):
                nc.vector.tensor_mul(at_sb[0:ME[n], n, :],
                                     at_ps[0:ME[n], :], brs[0:ME[n], :])

            # interleave deferred out-proj tiles to hide the recip latency
            a0 = av(0)
            if pending_out is not None:
                emit_tt(pending_out, 0, of_cur)
                emit_tt(pending_out, 1, of_cur)
            brs2 = brs_of(2)
            mul(2, at2, brs2)       # frees the at2 slot for a1
            if pending_out is not None:
                emit_tt(pending_out, 2, of_cur)
            brs0 = brs_of(0)
            brs1 = brs_of(1)
            a1 = av(1)
            mul(0, a0, brs0)
            mul(1, a1, brs1)
            if pending_out is not None:
                emit_tt(pending_out, 3, of_cur)
                emit_dma(pending_out, of_cur)
            pending_out = (b, c, at_sb)

        of_cur = ofp.tile([128, 4, CTX], BF16, tag="of")
        ob_l, oc_l, _ = pending_out
        for tt in range(4):
            emit_tt(pending_out, tt, of_cur)
            nc.sync.dma_start(
                out_e[ob_l, TCH * oc_l + 128 * tt:TCH * oc_l + 128 * tt + 128, :],
                of_cur[:, tt, :])

        mps.release()
        ap.release()
        ofp.release()
        mp.release()
        qtp.release()
        esp.release()
        cinp.release()
        pp.release()
        cp.release()

    split_waits(nc)
    return nc


def split_waits(nc, max_waits=1):
    """neuronxcc walrus accepts at most one attached sync wait per
    instruction; hoist extras onto standalone event-semaphore waits."""
    n_new = 0
    for f in nc.m.functions:
        for blk in f.blocks:
            new = []
            changed = False
            for inst in blk.instructions:
                si = inst.sync_info
                ow = list(si.on_wait) if (si is not None and si.on_wait) else []
                if len(ow) > max_waits:
                    for w in ow[:-max_waits]:
                        ev = mybir.InstEventSemaphore(
                            name=f"I-waitsplit-{n_new}", ins=[], outs=[])
                        ev.engine = inst.engine
                        ev.sync_info = mybir.SyncInfo(on_wait=[w], on_update=[])
                        nc.register_instruction(ev)
                        new.append(ev)
                        n_new += 1
                    inst.sync_info = mybir.SyncInfo(
                        on_wait=ow[-max_waits:], on_update=list(si.on_update))
                    changed = True
                new.append(inst)
            if changed:
                blk.instructions = new


_GRAPH = None


def _prep_in_maps(inputs):
    inputs = {k: np.asarray(v, dtype=np.float32) for k, v in inputs.items()}
    consts = _build_host_consts(inputs)
    ctx = np.asarray(inputs["context"])
    audio = np.asarray(inputs["audio_context"])

    # pack context as [B, NCH, 128p, 6n, TCH] so each chunk DMA is contiguous
    ctx16 = (ctx.transpose(0, 2, 1)
             .reshape(B, 6, 128, NCH, TCH)
             .transpose(0, 3, 2, 1, 4))
    ctx16 = np.ascontiguousarray(ctx16).astype(BF)
    apad = np.zeros((B, AUD + 2 * PAD), np.float32)
    apad[:, PAD:PAD + AUD] = audio[:, 0, :]
    a_im = np.empty((B, KS, AUD), np.float32)
    for k in range(KS):
        a_im[:, k, :] = apad[:, k:k + AUD]
    a_im = a_im.astype(BF)

    in_maps = []
    for core in range(NCORES):
        m = dict(consts)
        s = slice(core * BP, (core + 1) * BP)
        m["ctx16"] = ctx16[s]
        m["a_im"] = a_im[s]
        in_maps.append(m)
    return in_maps


def kernel(**inputs):
    global _GRAPH
    if _GRAPH is None:
        _GRAPH = _build_graph()
    nc = _GRAPH

    in_maps = _prep_in_maps(inputs)
    res = run_bass_kernel_spmd(nc, in_maps, list(range(NCORES)))
    out = np.concatenate([res.results[i]["out"] for i in range(NCORES)], axis=0)
    return out.astype(np.float32)



# revision 62
# speedup vs baseline: 1.0194x; 1.0194x over previous
"""Trainium2 Bass kernel for nn_Adapter (audio conv encoder + cross-attention), v5.

v4 baseline (278us) + pipeline/head/tail work (-> ~254us):
  - permuted e'-layout [c0: pair0+pair1a | c1: pair2+pair1b | c2: pair3]:
    sim 6->5 MMs, AV 9->8 MMs per chunk; denominators at psum rows 96:104
  - audio convs col-tiled; full audio (both batches) in the pre-phase,
    interleaved with 6 pre-emitted q-chunks (no audio in the main loop)
  - host-packed chunk-contiguous ctx ([B,NCH,128,6,TCH]) and pre-permuted
    wqt/wkt/wvt/woutA -> full-bandwidth DMA reads; small consts packed into
    blob DMAs; pre-phase cins on the in-order sync queue
  - deferred out-proj emitted per token-tile, interleaved into the next
    chunk's normalize chain (hides the Ln/Exp reciprocal latency);
    one merged out DMA per chunk; per-tt DMA on the final chunk
  - brp broadcast MMs on the "sp" psum slots (avoids waiting on out evacs);
    mul2 ordered right after brs2 so a1 reuses the at2 slot without stalls
  - ACT gelu table preloaded at t0; Ln/Exp table swap pinned after the
    conv1 gelus via a data dependency; xt via 4 full 128x128 PE transposes
"""
import sys
sys.path.insert(0, "/opt/trn_rl_repo")

import numpy as np
import ml_dtypes

import concourse.bass as bass
import concourse.mybir as mybir
import concourse.tile as tile
from concourse.bass_utils import run_bass_kernel_spmd

F32 = mybir.dt.float32
BF16 = mybir.dt.bfloat16
AF = mybir.ActivationFunctionType
BF = ml_dtypes.bfloat16

NCORES = 8
B, N, CTX = 16, 4096, 768
BP = B // NCORES
H, D, INNER = 8, 40, 320
AUD = 1024
KS, PAD = 17, 8
EPS = 1e-5
SCALE = D ** -0.5
TCH = 512
NCH = N // TCH
PADB = AUD + 2 * PAD

# e' permutation: slot -> old e (-1 = pad); bias at slot 352 (chunk2 row 96)
def _perm_new2old():
    p = [-1] * 384
    for s in range(128):
        p[s] = s                    # c0: pair0 (0:80) + pair1 dims 0:48
    for i in range(80):
        p[128 + i] = 160 + i        # c1 rows 0:80: pair2
    for i in range(32):
        p[208 + i] = 128 + i        # c1 rows 80:112: pair1 dims 48:80
    for i in range(80):
        p[256 + i] = 240 + i        # c2 rows 0:80: pair3
    return p


PN2O = _perm_new2old()
RN = [128, 112, 97]      # out-proj contraction rows (96=bias)
ME = [128, 112, 80]      # value rows per e'-chunk (normalize extent)


def _head_of_slot(n, r):
    o = PN2O[128 * n + r]
    return -1 if o < 0 else o // D


def _head_runs():
    """Contiguous (n, r0, r1, head) runs over the e'-chunks' value rows."""
    runs = []
    for n in range(3):
        r = 0
        while r < ME[n]:
            h = _head_of_slot(n, r)
            r1 = r
            while r1 < ME[n] and _head_of_slot(n, r1) == h:
                r1 += 1
            runs.append((n, r, r1, h))
            r = r1
    return runs


HEAD_RUNS = _head_runs()


def _build_host_consts(inputs):
    c = {}
    w1, b1 = inputs["w1"], inputs["b1"]
    w2, b2 = inputs["w2"], inputs["b2"]
    w3, b3 = inputs["w3"], inputs["b3"]
    c["w1t"] = np.ascontiguousarray(w1[:, 0, :].T).astype(BF)

    def pack_pairs(w):
        wp = np.zeros((128, 9, 64), np.float32)
        wt = w.transpose(1, 2, 0)
        for q in range(9):
            wp[0:64, q, :] = wt[:, 2 * q, :]
            if 2 * q + 1 < KS:
                wp[64:128, q, :] = wt[:, 2 * q + 1, :]
        return wp.astype(BF)

    c["w2p"] = pack_pairs(w2)
    c["w3p"] = pack_pairs(w3)

    def dup2(v):
        return np.tile(np.asarray(v), 2).reshape(128, 1).astype(np.float32)

    c["b1c2"] = dup2(b1)
    c["b2c2"] = dup2(b2)
    c["b3c2"] = dup2(b3)
    lnw = np.asarray(inputs["ln_w"]).astype(np.float32)
    lnb = np.asarray(inputs["ln_b"]).astype(np.float32)
    c["lnw2"] = np.concatenate([lnw[:, 0:512], lnw[:, 512:1024]], 0)
    c["lnb2"] = np.concatenate([lnb[:, 0:512], lnb[:, 512:1024]], 0)

    wq = np.asarray(inputs["wq"])
    wk = np.asarray(inputs["wk"])
    wv = np.asarray(inputs["wv"])
    wout = np.asarray(inputs["w_out"])
    bout = np.asarray(inputs["b_out"])
    wqt = np.zeros((CTX, 384), np.float32)
    wkt = np.zeros((AUD, 384), np.float32)
    wvt = np.zeros((AUD, 384), np.float32)
    wA = np.zeros((384, CTX), np.float32)
    for s, o in enumerate(PN2O):
        if o >= 0:
            wqt[:, s] = wq[o, :]
            wkt[:, s] = wk[o, :]
            wvt[:, s] = wv[o, :]
            wA[s] = wout[:, o]
    wA[352] = bout
    # pre-permute to [128p, n, e] so DMA reads are fully contiguous
    c["wqt"] = np.ascontiguousarray(
        wqt.reshape(6, 128, 384).transpose(1, 0, 2)).astype(BF)
    c["wkt"] = np.ascontiguousarray(
        wkt.reshape(8, 128, 384).transpose(1, 0, 2)).astype(BF)
    c["wvt"] = np.ascontiguousarray(
        wvt.reshape(8, 128, 384).transpose(1, 0, 2)).astype(BF)
    c["woutA"] = np.ascontiguousarray(
        wA.reshape(3, 128, CTX).transpose(1, 0, 2)).astype(BF)

    km_lo = np.zeros((80, 128), np.float32)
    km_lo[0:40, 0:64] = 1.0
    km_lo[40:80, 64:128] = 1.0
    c["kmask_lo"] = km_lo.astype(BF)
    km1a = np.zeros((128, 128), np.float32)
    km1a[80:120, 0:64] = 1.0
    km1a[120:128, 64:128] = 1.0
    c["kmask1a"] = km1a.astype(BF)
    km1b = np.zeros((128, 128), np.float32)
    km1b[80:112, 64:128] = 1.0
    c["kmask1b"] = km1b.astype(BF)

    vm = np.zeros((128, 80), np.float32)
    vm[0:64, 0:40] = 1.0
    vm[64:128, 40:80] = 1.0
    c["vmask_v"] = vm.astype(BF)

    # exp8': [104, 3, 128] head->row broadcast selector (rows 96:104 used)
    e8 = np.zeros((104, 3, 128), np.float32)
    for n in range(3):
        for r in range(ME[n]):
            h = _head_of_slot(n, r)
            if h >= 0:
                e8[96 + h, n, r] = 1.0
    c["exp8"] = e8.astype(BF)

    # mini-blobs: conv1 weights (bf16) + the three biases (f32)
    mini = np.zeros((17, 64), BF)
    mini[:, :] = c["w1t"]
    c["mini"] = mini
    minif = np.concatenate([c.pop("b1c2"), c.pop("b2c2"), c.pop("b3c2")], 1)
    c["minif"] = np.ascontiguousarray(minif)
    # pack small consts into two blobs: one DMA each at kernel start
    bb = np.zeros((128, 2064), BF)
    bb[0:17, 0:64] = c.pop("w1t")
    bb[:, 64:640] = c.pop("w2p").reshape(128, 576)
    bb[:, 640:1216] = c.pop("w3p").reshape(128, 576)
    bb[0:80, 1216:1344] = c.pop("kmask_lo")
    bb[:, 1344:1472] = c.pop("kmask1a")
    bb[:, 1472:1600] = c.pop("kmask1b")
    bb[:, 1600:1680] = c.pop("vmask_v")
    bb[0:104, 1680:2064] = c.pop("exp8").reshape(104, 384)
    c["blob_bf"] = bb
    bf = np.zeros((128, 1155), np.float32)
    bf[:, 3:515] = c.pop("lnw2")
    bf[:, 515:1027] = c.pop("lnb2")
    bf[:, 1027:1155] = np.eye(128, dtype=np.float32)
    c["blob_f"] = bf
    return c


def _build_graph():
    nc = bass.Bass()
    P = {}

    def inp(name, shape, dt):
        P[name] = nc.declare_dram_parameter(name, list(shape), dt, isOutput=False)

    inp("ctx16", (BP, NCH, 128, 6, TCH), BF16)
    inp("a_im", (BP, KS, AUD), BF16)
    inp("blob_bf", (128, 2064), BF16)
    inp("blob_f", (128, 1155), F32)
    inp("wqt", (128, 6, 384), BF16)
    inp("wkt", (128, 8, 384), BF16)
    inp("wvt", (128, 8, 384), BF16)
    inp("woutA", (128, 3, CTX), BF16)
    out_e = nc.declare_dram_parameter("out", [BP, N, CTX], BF16, isOutput=True)

    with tile.TileContext(nc) as tc:
        cp = tc.alloc_tile_pool(name="const", bufs=1)
        pp = tc.alloc_tile_pool(name="persist", bufs=1)
        cinp = tc.alloc_tile_pool(name="cinp", bufs=7)
        esp = tc.alloc_tile_pool(name="esp", bufs=6)
        qtp = tc.alloc_tile_pool(name="qtp", bufs=6)
        mp = tc.alloc_tile_pool(name="mp", bufs=2)
        ofp = tc.alloc_tile_pool(name="ofp", bufs=2)
        ap = tc.alloc_tile_pool(name="audio", bufs=1)
        aps = tc.alloc_tile_pool(name="aps", bufs=2, space="PSUM")

        def cload(name, shape, dt, ap_src=None):
            t = cp.tile(list(shape), dt, tag=name)
            nc.sync.dma_start(t[:], ap_src if ap_src is not None else P[name][:])
            return t

        # ---- t0: trigger the gelu ACT-table load while DMAs stream ----
        tiny = cp.tile([1, 2], F32, tag="tiny")
        nc.vector.memset(tiny[:], 0.0)
        tinyg = cp.tile([1, 2], F32, tag="tinyg")
        nc.scalar.activation(tinyg[:], tiny[:], AF.Gelu)

        # ---- mini-blob (conv1 weights + biases) first, audio inputs,
        # ---- then one blob DMA for the remaining small consts ----
        mini = cp.tile([KS, 64], BF16, tag="mini")
        nc.sync.dma_start(mini[:], P["mini"][:])
        w1t = mini[:]
        minif = cp.tile([128, 3], F32, tag="minif")
        nc.sync.dma_start(minif[:], P["minif"][:])
        b1c2 = minif[:, 0:1]
        b2c2 = minif[:, 1:2]
        b3c2 = minif[:, 2:3]

        a_sbs = {}
        for b in range(BP):
            a_sb = ap.tile([KS, AUD], BF16, tag=f"a_im{b}")
            nc.sync.dma_start(a_sb[:], P["a_im"][b])
            a_sbs[b] = a_sb

        blob_bf = cp.tile([128, 2064], BF16, tag="blob_bf")
        nc.sync.dma_start(blob_bf[:], P["blob_bf"][:])
        w2p = blob_bf[:, 64:640].rearrange("p (a b) -> p a b", a=9)
        w3p = blob_bf[:, 640:1216].rearrange("p (a b) -> p a b", a=9)
        kmask_lo = blob_bf[0:80, 1216:1344]
        kmask1a = blob_bf[:, 1344:1472]
        kmask1b = blob_bf[:, 1472:1600]
        vmask_v = blob_bf[:, 1600:1680]
        exp8 = blob_bf[0:104, 1680:2064].rearrange("p (a b) -> p a b", a=3)

        blob_f = cp.tile([128, 1155], F32, tag="blob_f")
        nc.sync.dma_start(blob_f[:], P["blob_f"][:])
        lnw2 = blob_f[:, 3:515]
        lnb2 = blob_f[:, 515:1027]
        ident128 = blob_f[:, 1027:1155]

        ones128 = cp.tile([128, 128], BF16, tag="ones128")
        nc.vector.memset(ones128[:], 1.0)

        # ---- audio encoder phases (dual-row [128, 512] layout) ----
        xb2s, x2bs, statss, xb3s, x_sbs, xts = {}, {}, {}, {}, {}, {}
        g1s = {}
        kp_all, vp_all = [None, None], [None, None]

        def asm_dual(dst, g):
            nc.gpsimd.memset(dst[0:64, 0:PAD], 0.0)
            nc.gpsimd.memset(dst[0:64, AUD + PAD:PADB], 0.0)
            nc.gpsimd.memset(dst[64:128, 0:PAD - 1], 0.0)
            nc.gpsimd.memset(dst[64:128, PAD + AUD - 1:PADB], 0.0)
            nc.scalar.dma_start(dst[0:64, PAD:PAD + 512], g[0:64, :])
            nc.scalar.dma_start(dst[0:64, PAD + 512:PAD + 1024], g[64:128, :])
            nc.scalar.dma_start(dst[64:128, PAD - 1:PAD + 511], g[0:64, :])
            nc.scalar.dma_start(dst[64:128, PAD + 511:PAD + 1023], g[64:128, :])

        def conv_ct(psp, cvtag, wtile, src):
            cv = psp.tile([128, 512], F32, tag=cvtag)
            for q in range(9):
                nc.tensor.matmul(cv[0:64, :], wtile[:, q, :],
                                 src[:, 2 * q:2 * q + 512],
                                 start=(q == 0), stop=(q == 8),
                                 tile_position=(0, 0))
                nc.tensor.matmul(cv[64:128, :], wtile[:, q, :],
                                 src[:, 2 * q + 512:2 * q + 1024],
                                 start=(q == 0), stop=(q == 8),
                                 tile_position=(0, 64))
            return cv

        def ph_conv1(b, psp, cvtag):
            cv1 = psp.tile([128, 512], F32, tag=cvtag)
            nc.tensor.matmul(cv1[0:64, :], w1t[:], a_sbs[b][:, 0:512],
                             start=True, stop=True, tile_position=(0, 0))
            nc.tensor.matmul(cv1[64:128, :], w1t[:], a_sbs[b][:, 512:1024],
                             start=True, stop=True, tile_position=(0, 64))
            g1 = ap.tile([128, 512], BF16, tag=f"g1{b}")
            nc.scalar.activation(g1[:], cv1[:], AF.Gelu, bias=b1c2[:])
            g1s[b] = g1
            xb2 = ap.tile([128, PADB], BF16, tag=f"xb2{b}")
            asm_dual(xb2, g1)
            xb2s[b] = xb2

        def ph_conv2(b, psp, cvtag):
            cv2 = conv_ct(psp, cvtag, w2p, xb2s[b])
            x2b = ap.tile([128, 512], F32, tag=f"x2b{b}")
            stats = ap.tile([128, 2], F32, tag=f"stats{b}")
            sq = ap.tile([128, 512], F32, tag=f"sq{b}")
            nc.vector.tensor_scalar(
                out=x2b[:], in0=cv2[:], scalar1=b2c2[:], scalar2=0.0,
                op0=mybir.AluOpType.add, op1=mybir.AluOpType.add,
                accum_out=stats[:, 0:1])
            nc.vector.tensor_mul(sq[:], x2b[:], x2b[:])
            nc.vector.reduce_sum(stats[:, 1:2], sq[:], axis=mybir.AxisListType.X)
            x2bs[b] = x2b
            statss[b] = stats

        def ph_ln(b, psp, cvtag):
            stats = statss[b]
            x2b = x2bs[b]
            st16 = ap.tile([128, 2], BF16, tag=f"st16{b}")
            nc.vector.tensor_copy(st16[:], stats[:])
            totp = psp.tile([128, 64], F32, tag=cvtag)
            nc.tensor.matmul(totp[:, 0:2], ones128[:], st16[:], start=True, stop=True)

            mu = ap.tile([128, 1], F32, tag=f"mu{b}")
            msq = ap.tile([128, 1], F32, tag=f"msq{b}")
            var = ap.tile([128, 1], F32, tag=f"var{b}")
            sd = ap.tile([128, 1], F32, tag=f"sd{b}")
            rstd = ap.tile([128, 1], F32, tag=f"rstd{b}")
            nmr = ap.tile([128, 1], F32, tag=f"nmr{b}")
            inv_n = 1.0 / (64 * AUD)
            nc.vector.tensor_scalar_mul(mu[:], totp[:, 0:1], inv_n)
            nc.vector.tensor_scalar_mul(msq[:], totp[:, 1:2], inv_n)
            nc.vector.tensor_mul(var[:], mu[:], mu[:])
            nc.vector.tensor_sub(var[:], msq[:], var[:])
            nc.vector.tensor_scalar_add(var[:], var[:], EPS)
            nc.scalar.activation(sd[:], var[:], AF.Ln)
            nc.scalar.activation(rstd[:], sd[:], AF.Exp, scale=-0.5)
            nc.vector.tensor_mul(nmr[:], mu[:], rstd[:])
            nc.vector.tensor_scalar_mul(nmr[:], nmr[:], -1.0)

            t1 = ap.tile([128, 512], F32, tag=f"t1{b}")
            t2 = ap.tile([128, 512], F32, tag=f"t2{b}")
            g3 = ap.tile([128, 512], BF16, tag=f"g3{b}")
            nc.vector.tensor_scalar(out=t1[:], in0=x2b[:], scalar1=rstd[:],
                                    scalar2=nmr[:], op0=mybir.AluOpType.mult,
                                    op1=mybir.AluOpType.add)
            nc.vector.tensor_mul(t2[:], t1[:], lnw2[:])
            nc.vector.tensor_add(g3[:], t2[:], lnb2[:])
            xb3 = ap.tile([128, PADB], BF16, tag=f"xb3{b}")
            asm_dual(xb3, g3)
            xb3s[b] = xb3

        def ph_conv3(b, psp, cvtag):
            cv3 = conv_ct(psp, cvtag, w3p, xb3s[b])
            x_sb = ap.tile([128, 512], F32, tag=f"x_sb{b}")
            nc.vector.tensor_scalar(
                out=x_sb[:], in0=cv3[:], scalar1=b3c2[:], scalar2=0.0,
                op0=mybir.AluOpType.add, op1=mybir.AluOpType.add)
            x_sbs[b] = x_sb

        def ph_xt(b, psp, cvtag):
            # one [128,128] transpose yields both L-halves' [128L, 64ch] tiles
            xt = pp.tile([128, 8, 64], BF16, tag=f"xt{b}")
            for f in range(4):
                pt = psp.tile([128, 128], F32, tag=cvtag)
                nc.tensor.transpose(pt[:], x_sbs[b][:, 128 * f:128 * f + 128],
                                    ident128[:])
                nc.scalar.activation(xt[:, f::4, :], pt[:], AF.Copy)
            xts[b] = xt

        def ph_ktv(b, psp, cvtag):
            xt = xts[b]
            kt = pp.tile([128, 3, 64], BF16, tag=f"kt{b}")
            for m in range(3):
                ktp = psp.tile([128, 64], F32, tag=cvtag)
                for aj in range(8):
                    nc.tensor.matmul(ktp[:], wkt[:, aj, 128 * m:128 * m + 128],
                                     xt[:, aj, :], start=(aj == 0), stop=(aj == 7))
                nc.scalar.activation(kt[:, m, :], ktp[:], AF.Copy)

            v2p = psp.tile([128, 384], F32, tag=cvtag)
            for aj in range(8):
                nc.tensor.matmul(v2p[0:64, :], xt[:, aj, :], wvt[:, aj, :],
                                 start=(aj == 0), stop=(aj == 7),
                                 tile_position=(0, 0))
                nc.tensor.matmul(v2p[64:128, :], xt[:, aj, :], wvt[:, aj, :],
                                 start=(aj == 0), stop=(aj == 7),
                                 tile_position=(0, 64))
            v2 = pp.tile([128, 384], BF16, tag=f"v2{b}")
            nc.scalar.activation(v2[:], v2p[:], AF.Copy)

            # kp statics for sim (5 MMs/chunk)
            def mk_kp80(tag, ktsl):
                t = pp.tile([80, 128], BF16, tag=tag)
                nc.vector.tensor_mul(
                    t[:].rearrange("p (a j) -> p a j", a=2),
                    ktsl.broadcast_to([80, 2, 64]),
                    kmask_lo[:].rearrange("p (a j) -> p a j", a=2))
                return t

            kp0 = mk_kp80(f"kp0_{b}", kt[0:80, 0:1, :])
            kp2 = mk_kp80(f"kp2_{b}", kt[0:80, 1:2, :])
            kp3 = mk_kp80(f"kp3_{b}", kt[0:80, 2:3, :])
            kp1a = pp.tile([128, 128], BF16, tag=f"kp1a_{b}")
            nc.vector.tensor_mul(
                kp1a[64:128, :].rearrange("p (a j) -> p a j", a=2),
                kt[64:128, 0:1, :].broadcast_to([64, 2, 64]),
                kmask1a[64:128, :].rearrange("p (a j) -> p a j", a=2))
            kp1b = pp.tile([128, 128], BF16, tag=f"kp1b_{b}")
            nc.vector.tensor_mul(
                kp1b[64:128, :].rearrange("p (a j) -> p a j", a=2),
                kt[64:128, 1:2, :].broadcast_to([64, 2, 64]),
                kmask1b[64:128, :].rearrange("p (a j) -> p a j", a=2))
            kp_all[b] = (kp0, kp1a, kp1b, kp2, kp3)

            # vp value statics (baseline-style per (chunk, pair))
            vps = {}
            vp = pp.tile([128, 128], BF16, tag=f"vp00_{b}")     # (0, p0)
            nc.gpsimd.memset(vp[:, 80:128], 0.0)
            nc.vector.tensor_mul(vp[:, 0:80], v2[:, 0:80], vmask_v[:])
            vps[(0, 0)] = vp
            vp = pp.tile([128, 128], BF16, tag=f"vp01_{b}")     # (0, p1) dims 0:48
            nc.gpsimd.memset(vp[:, 0:80], 0.0)
            nc.vector.tensor_mul(vp[:, 80:128], v2[:, 80:128], vmask_v[:, 0:48])
            vps[(0, 1)] = vp
            vp = pp.tile([128, 112], BF16, tag=f"vp12_{b}")     # (1, p2)
            nc.gpsimd.memset(vp[:, 80:112], 0.0)
            nc.vector.tensor_mul(vp[:, 0:80], v2[:, 128:208], vmask_v[:])
            vps[(1, 2)] = vp
            vp = pp.tile([128, 112], BF16, tag=f"vp11_{b}")     # (1, p1) dims 48:80
            nc.gpsimd.memset(vp[:, 0:80], 0.0)
            nc.vector.tensor_mul(vp[:, 80:112], v2[:, 208:240], vmask_v[:, 48:80])
            vps[(1, 1)] = vp
            vp = pp.tile([128, 104], BF16, tag=f"vp23_{b}")     # (2, p3) + denoms
            nc.gpsimd.memset(vp[:, 80:104], 0.0)
            nc.vector.tensor_mul(vp[:, 0:80], v2[:, 256:336], vmask_v[:])
            nc.gpsimd.memset(vp[0:64, 102:103], 1.0)
            nc.gpsimd.memset(vp[64:128, 103:104], 1.0)
            vps[(2, 3)] = vp
            vp_all[b] = vps

        # denominator-only statics for at2 (batch-independent)
        vpd = []
        for p in range(3):
            t = cp.tile([128, 104], BF16, tag=f"vpd{p}")
            nc.gpsimd.memset(t[:], 0.0)
            nc.gpsimd.memset(t[0:64, 96 + 2 * p:97 + 2 * p], 1.0)
            nc.gpsimd.memset(t[64:128, 97 + 2 * p:98 + 2 * p], 1.0)
            vpd.append(t)

        # ---- attention loads by need ----
        wqt = cload("wqt", (128, 6, 384), BF16)
        cin_pre = {}
        for c0_ in range(2):
            t = cinp.tile([128, 6, TCH], BF16, tag="cin")
            nc.sync.dma_start(t[:], P["ctx16"][0, c0_])
            cin_pre[(0, c0_)] = t
        wkt = cload("wkt", (128, 8, 384), BF16)
        wvt = cload("wvt", (128, 8, 384), BF16)
        for c0_ in range(2, 4):
            t = cinp.tile([128, 6, TCH], BF16, tag="cin")
            nc.sync.dma_start(t[:], P["ctx16"][0, c0_])
            cin_pre[(0, c0_)] = t
        woutA = cload("woutA", (128, 3, CTX), BF16)
        for c0_ in range(4, 6):
            t = cinp.tile([128, 6, TCH], BF16, tag="cin")
            nc.sync.dma_start(t[:], P["ctx16"][0, c0_])
            cin_pre[(0, c0_)] = t

        def emit_q(cin, psum_pool, psum_tag):
            qt = qtp.tile([128, 3, TCH], BF16, tag="qt")
            for m in range(3):
                qp = psum_pool.tile([128, TCH], F32, tag=psum_tag)
                for n6 in range(6):
                    nc.tensor.matmul(qp[:], wqt[:, n6, 128 * m:128 * m + 128],
                                     cin[:, n6, :], start=(n6 == 0), stop=(n6 == 5))
                nc.vector.tensor_copy(qt[:, m, :], qp[:])
            return qt

        # ---- pre-phase: full audio for both batches + first 4 q-emits ----
        ph_conv1(0, aps, "cv")
        ph_conv1(1, aps, "cv")
        # after both gelus: swap the ACT table to natural_log_exp.
        # reads g1(b1) so the scheduler cannot hoist it before the gelus.
        tinyl = cp.tile([1, 2], F32, tag="tinyl")
        nc.scalar.activation(tinyl[:], g1s[1][0:1, 0:2], AF.Ln)

        qt_pre = {}
        qt_pre[0] = emit_q(cin_pre[(0, 0)], aps, "qpre")
        qt_pre[1] = emit_q(cin_pre[(0, 1)], aps, "qpre")
        ph_conv2(0, aps, "cv")
        ph_conv2(1, aps, "cv")
        qt_pre[2] = emit_q(cin_pre[(0, 2)], aps, "qpre")
        ph_ln(0, aps, "cv")
        ph_ln(1, aps, "cv")
        ph_conv3(0, aps, "cv")
        ph_conv3(1, aps, "cv")
        qt_pre[3] = emit_q(cin_pre[(0, 3)], aps, "qpre")
        ph_xt(0, aps, "cv")
        ph_xt(1, aps, "cv")
        ph_ktv(0, aps, "cv")
        qt_pre[4] = emit_q(cin_pre[(0, 4)], aps, "qpre")
        ph_ktv(1, aps, "cv")
        qt_pre[5] = emit_q(cin_pre[(0, 5)], aps, "qpre")

        aps.release()

        # ---- main attention loop ----
        mps = tc.alloc_tile_pool(name="mps", bufs=2, space="PSUM")

        at_sbs = []
        for k2 in range(2):
            t = pp.tile([128, 3, TCH], BF16, tag=f"at_sb{k2}")
            nc.gpsimd.memset(t[64:96, 2, :], 0.0)
            nc.gpsimd.memset(t[96:97, 2, :], 1.0)
            at_sbs.append(t)

        pending_out = None

        def emit_tt(job, tt, of):
            ob, oc, oat = job
            for ci, (c0, cw) in enumerate(((0, 384), (384, 384))):
                op = mps.tile([128, 512], F32, tag="ob")
                for n in range(3):
                    nc.tensor.matmul(
                        op[:, 0:cw],
                        oat[0:RN[n], n, 128 * tt:128 * tt + 128],
                        woutA[0:RN[n], n, c0:c0 + cw],
                        start=(n == 0), stop=(n == 2))
                if ci == 0:
                    nc.scalar.activation(of[:, tt, c0:c0 + cw], op[:, 0:cw],
                                         AF.Copy)
                else:
                    nc.vector.tensor_copy(of[:, tt, c0:c0 + cw], op[:, 0:cw])

        def emit_dma(job, of):
            ob, oc, oat = job
            nc.sync.dma_start(
                out_e[ob, TCH * oc:TCH * oc + TCH, :]
                .rearrange("(a p) c -> p a c", p=128), of[:])

        chunks = [(bb, cc2) for bb in range(BP) for cc2 in range(NCH)]
        cins = dict(cin_pre)
        qts = {i2: qt_pre[i2] for i2 in range(len(qt_pre))}

        for i, (b, c) in enumerate(chunks):
            kp0, kp1a, kp1b, kp2, kp3 = kp_all[b]
            vps = vp_all[b]
            for la in (3, 4):
                if i + la < len(chunks) and chunks[i + la] not in cins:
                    b3, c3 = chunks[i + la]
                    t = cinp.tile([128, 6, TCH], BF16, tag="cin")
                    nc.gpsimd.dma_start(t[:], P["ctx16"][b3, c3])
                    cins[chunks[i + la]] = t

            qt = qts.pop(i)
            of_cur = (ofp.tile([128, 4, CTX], BF16, tag="of", name="of_cur")
                      if pending_out is not None else None)

            sim_defs = [
                [(kp0[:], qt[0:80, 0, :], None)],
                [(kp1a[64:128, :], qt[64:128, 0, :], (64, 0)),
                 (kp1b[64:128, :], qt[64:128, 1, :], (64, 0))],
                [(kp2[:], qt[0:80, 1, :], None)],
                [(kp3[:], qt[0:80, 2, :], None)],
            ]
            es = [None] * 4

            def sim(p):
                sp = mps.tile([128, TCH], F32, tag="sp")
                plan = sim_defs[p]
                for ii, (lh, rh, tp) in enumerate(plan):
                    nc.tensor.matmul(sp[:], lh, rh, start=(ii == 0),
                                     stop=(ii == len(plan) - 1),
                                     tile_position=tp)
                e = esp.tile([128, TCH], BF16, tag="es")
                nc.scalar.activation(e[:], sp[:], AF.Exp, scale=SCALE)
                es[p] = e

            sim(0)
            sim(1)
            # q for chunk i+2 fills the PE while the first exps run on ACT
            if i + 2 < len(chunks) and (i + 2) not in qts:
                qts[i + 2] = emit_q(cins.pop(chunks[i + 2]), mps, "qp")
            sim(2)
            sim(3)

            at_sb = at_sbs[i % 2]

            # at2: chunk2 AV (pair3) + all denominators at rows 96:104
            at2 = mps.tile([104, TCH], F32, tag="at")
            at2_ops = [(vpd[0], 0), (vpd[1], 1), (vpd[2], 2), (vps[(2, 3)], 3)]
            for ii, (vpt, p) in enumerate(at2_ops):
                nc.tensor.matmul(at2[:], vpt[:, 0:104], es[p][:],
                                 start=(ii == 0), stop=(ii == 3))
            lnd = mp.tile([104, TCH], F32, tag="lnd")
            rec16 = mp.tile([104, TCH], BF16, tag="rec16")
            nc.scalar.activation(lnd[96:104, :], at2[96:104, :], AF.Ln)
            nc.scalar.activation(rec16[96:104, :], lnd[96:104, :], AF.Exp,
                                 scale=-1.0)

            def brs_of(n):
                brp = mps.tile([128, TCH], F32, tag="sp")
                nc.tensor.matmul(brp[0:ME[n], :], exp8[96:104, n, 0:ME[n]],
                                 rec16[96:104, :], start=True, stop=True,
                                 tile_position=(96, 0))
                brs = mp.tile([128, TCH], BF16, tag="brs", bufs=3)
                nc.vector.tensor_copy(brs[0:ME[n], :], brp[0:ME[n], :])
                return brs

            def av(n):
                a = mps.tile([128, TCH], F32, tag="at")
                W = 128 if n == 0 else 112
                prs = [(0, 0), (0, 1)] if n == 0 else [(1, 2), (1, 1)]
                for ii, key in enumerate(prs):
                    nc.tensor.matmul(a[0:W, :], vps[key][:], es[key[1]][:],
                                     start=(ii == 0), stop=(ii == 1))
                return a

            def mul(n, at_ps, brs):
                nc.vector.tensor_mul(at_sb[0:ME[n], n, :],
                                     at_ps[0:ME[n], :], brs[0:ME[n], :])

            # interleave deferred out-proj tiles to hide the recip latency
            a0 = av(0)
            if pending_out is not None:
                emit_tt(pending_out, 0, of_cur)
                emit_tt(pending_out, 1, of_cur)
            brs2 = brs_of(2)
            mul(2, at2, brs2)       # frees the at2 slot for a1
            if pending_out is not None:
                emit_tt(pending_out, 2, of_cur)
            brs0 = brs_of(0)
            brs1 = brs_of(1)
            a1 = av(1)
            mul(0, a0, brs0)
            mul(1, a1, brs1)
            if pending_out is not None:
                emit_tt(pending_out, 3, of_cur)
                emit_dma(pending_out, of_cur)
            pending_out = (b, c, at_sb)

        of_cur = ofp.tile([128, 4, CTX], BF16, tag="of")
        ob_l, oc_l, _ = pending_out
        for tt in range(4):
            emit_tt(pending_out, tt, of_cur)
            nc.sync.dma_start(
                out_e[ob_l, TCH * oc_l + 128 * tt:TCH * oc_l + 128 * tt + 128, :],
                of_cur[:, tt, :])

        mps.release()
        ap.release()
        ofp.release()
        mp.release()
        qtp.release()
        esp.release()
        cinp.release()
        pp.release()
        cp.release()

    split_waits(nc)
    return nc


def split_waits(nc, max_waits=1):
    """neuronxcc walrus accepts at most one attached sync wait per
    instruction; hoist extras onto standalone event-semaphore waits."""
    n_new = 0
    for f in nc.m.functions:
        for blk in f.blocks:
            new = []
            changed = False
            for inst in blk.instructions:
                si = inst.sync_info
                ow = list(si.on_wait) if (si is not None and si.on_wait) else []
                if len(ow) > max_waits:
                    for w in ow[:-max_waits]:
                        ev = mybir.InstEventSemaphore(
                            name=f"I-waitsplit-{n_new}", ins=[], outs=[])
                        ev.engine = inst.engine
                        ev.sync_info = mybir.SyncInfo(on_wait=[w], on_update=[])
                        nc.register_instruction(ev)
                        new.append(ev)
                        n_new += 1
                    inst.sync_info = mybir.SyncInfo(
                        on_wait=ow[-max_waits:], on_update=list(si.on_update))
                    changed = True
                new.append(inst)
            if changed:
                blk.instructions = new


_GRAPH = None


def _prep_in_maps(inputs):
    inputs = {k: np.asarray(v, dtype=np.float32) for k, v in inputs.items()}
    consts = _build_host_consts(inputs)
    ctx = np.asarray(inputs["context"])
    audio = np.asarray(inputs["audio_context"])

    # pack context as [B, NCH, 128p, 6n, TCH] so each chunk DMA is contiguous
    ctx16 = (ctx.transpose(0, 2, 1)
             .reshape(B, 6, 128, NCH, TCH)
             .transpose(0, 3, 2, 1, 4))
    ctx16 = np.ascontiguousarray(ctx16).astype(BF)
    apad = np.zeros((B, AUD + 2 * PAD), np.float32)
    apad[:, PAD:PAD + AUD] = audio[:, 0, :]
    a_im = np.empty((B, KS, AUD), np.float32)
    for k in range(KS):
        a_im[:, k, :] = apad[:, k:k + AUD]
    a_im = a_im.astype(BF)

    in_maps = []
    for core in range(NCORES):
        m = dict(consts)
        s = slice(core * BP, (core + 1) * BP)
        m["ctx16"] = ctx16[s]
        m["a_im"] = a_im[s]
        in_maps.append(m)
    return in_maps


def kernel(**inputs):
    global _GRAPH
    if _GRAPH is None:
        _GRAPH = _build_graph()
    nc = _GRAPH

    in_maps = _prep_in_maps(inputs)
    res = run_bass_kernel_spmd(nc, in_maps, list(range(NCORES)))
    out = np.concatenate([res.results[i]["out"] for i in range(NCORES)], axis=0)
    return out.astype(np.float32)

